# revision 25
# baseline (speedup 1.0000x reference)
"""Fused self-attention (QKV projection + softmax attention) on 8 trn2 cores.

Problem shapes: t [4, 2048, 1024] f32, W_qkv [3072, 1024], b_qkv [3072].
out = softmax((t@Wq.T+bq) @ (t@Wk.T+bk).T / sqrt(1024)) @ (t@Wv.T+bv),
per batch; output [4, 2048, 1024] f32.

Sharding: 8 cores = (batch b in 0..3) x (query-half h in 0..1). Each core:
  - receives t[b].T restricted to its own query-half columns,
  - projects Q^T, and K^T / V for its own 1024 keys only,
  - pairwise-AllGathers K^T then V with its batch partner (keys end up in
    natural order on both cores; the program stays rank-agnostic),
  - S^T = K @ Q^T over all 2048 keys, E^T = exp(S^T) (no max subtraction:
    |logits| < ~6 for this input distribution),
  - out = (E^T).T @ [V | ones] -> unnormalized out + row-sum D,
  - out = out * (1/D) + bv.
All matmuls bf16 with fp32 PSUM accumulation; the 1/sqrt(d_k) scale is folded
into Wq/bq on the host. Per-core matmul work is 15.05 GFLOP = total/8, the
parallel minimum; the two collectives overlap with Q projection and scores.

Measured (8x trn2, axon): ~250 us per execution (12-rep vs 4-rep NEFF
marginal, dispatch-cancelled), rel err vs fp32 reference 0.46%. The 896
N=512 matmuls at the hardware's measured 254.5 ns/MM envelope give a
~228 us floor for this decomposition; collectives and all DVE/ACT/DMA
work are hidden under the PE stream.
"""

import math
import os
from contextlib import ExitStack

import numpy as np
import ml_dtypes

import concourse.bass as bass
import concourse.tile as tile
from concourse import bacc, mybir
from concourse.bass_utils import run_bass_kernel_spmd

P = 128
D = 1024          # d_model = d_k = d_v
NKEYS = 2048      # keys per batch (after gather)
NOWN = 1024       # keys projected per core
NQ = 1024         # queries per core
DT = D // P       # 8 contraction tiles
NT = NKEYS // P   # 16 key tiles
QT = NQ // P      # 8 query tiles
CH = 512          # moving-operand chunk (one PSUM bank of fp32)
BF = mybir.dt.bfloat16
F32 = mybir.dt.float32
AF = mybir.ActivationFunctionType
GROUPS = [[0, 1], [2, 3], [4, 5], [6, 7]]

_CACHE = {}
LAST_RESULTS = None


def _build_nc(n_reps=1, no_cc=False):
    nc = bacc.Bacc("TRN2", target_bir_lowering=False, debug=False, num_devices=8)

    tq_d = nc.dram_tensor("tq", [D, NOWN], BF, kind="ExternalInput").ap()
    wqT_d = nc.dram_tensor("wqT", [D, D], BF, kind="ExternalInput").ap()
    wkT_d = nc.dram_tensor("wkT", [D, D], BF, kind="ExternalInput").ap()
    wvT_d = nc.dram_tensor("wvT", [D, D], BF, kind="ExternalInput").ap()
    bq_d = nc.dram_tensor("bq", [DT, P], F32, kind="ExternalInput").ap()
    bk_d = nc.dram_tensor("bk", [DT, P], F32, kind="ExternalInput").ap()
    bv_d = nc.dram_tensor("bv", [D], F32, kind="ExternalInput").ap()
    out_d = nc.dram_tensor("out", [NQ, D], F32, kind="ExternalOutput").ap()

    with tile.TileContext(nc) as tc, ExitStack() as ctx:
        consts = ctx.enter_context(tc.tile_pool(name="consts", bufs=1))
        p_rd = ctx.enter_context(tc.tile_pool(name="p_rd", bufs=2))
        p_t = ctx.enter_context(tc.tile_pool(name="p_t", bufs=2))
        p_w = ctx.enter_context(tc.tile_pool(name="p_w", bufs=3))
        p_tmp = ctx.enter_context(tc.tile_pool(name="p_tmp", bufs=2))
        p_kt = ctx.enter_context(tc.tile_pool(name="p_kt", bufs=1))
        p_qt = ctx.enter_context(tc.tile_pool(name="p_qt", bufs=1))
        p_v = ctx.enter_context(tc.tile_pool(name="p_v", bufs=1))
        p_out = ctx.enter_context(tc.tile_pool(name="p_out", bufs=2))
        p_ps = ctx.enter_context(tc.tile_pool(name="p_ps", bufs=6, space="PSUM"))
        p_psd = ctx.enter_context(tc.tile_pool(name="p_psd", bufs=2, space="PSUM"))
        dram = ctx.enter_context(tc.tile_pool(name="dram", bufs=1, space="DRAM"))

        # ---- constants (loaded once) ----
        bq_sb = consts.tile([P, DT], F32, tag="bq")
        nc.sync.dma_start(out=bq_sb, in_=bq_d.rearrange("a p -> p a"))
        bk_sb = consts.tile([P, DT], F32, tag="bk")
        nc.sync.dma_start(out=bk_sb, in_=bk_d.rearrange("a p -> p a"))
        bv_sb = consts.tile([P, D], F32, tag="bv")
        nc.sync.dma_start(
            out=bv_sb,
            in_=bass.AP(tensor=bv_d.tensor, offset=bv_d.offset,
                        ap=[[0, P]] + list(bv_d.ap)),
        )
        ones_sb = consts.tile([P, 1], BF, tag="ones")
        nc.vector.memset(ones_sb, 1.0)

        for _rep in range(n_reps):
            _emit_body(nc, tc, locals(), no_cc=no_cc)

    nc.compile()
    return nc


def _emit_body(nc, tc, env, no_cc=False):
    consts = env["consts"]; p_rd = env["p_rd"]; p_t = env["p_t"]
    p_w = env["p_w"]; p_tmp = env["p_tmp"]; p_kt = env["p_kt"]
    p_qt = env["p_qt"]; p_v = env["p_v"]; p_out = env["p_out"]
    p_ps = env["p_ps"]; p_psd = env["p_psd"]; dram = env["dram"]
    bq_sb = env["bq_sb"]; bk_sb = env["bk_sb"]; bv_sb = env["bv_sb"]
    ones_sb = env["ones_sb"]
    tq_d = env["tq_d"]; wqT_d = env["wqT_d"]; wkT_d = env["wkT_d"]
    wvT_d = env["wvT_d"]; out_d = env["out_d"]

    if True:
        cc_in_k = dram.tile([D, NOWN], BF, tag="cik", name="cc_in_k")
        cc_out_k = dram.tile([2, D, NOWN], BF, tag="cok", name="cc_out_k")
        cc_in_v = dram.tile([NOWN, D], BF, tag="civ", name="cc_in_v")
        cc_out_v = dram.tile([2, NOWN, D], BF, tag="cov", name="cc_out_v")

        # ---- input loads ----
        tq = p_t.tile([P, DT, NOWN], BF, tag="tq", name="tq")
        tq_r = tq_d.rearrange("(dt p) n -> dt p n", p=P)
        for dt in range(DT):
            nc.sync.dma_start(out=tq[:, dt, :], in_=tq_r[dt])

        ws = {}
        for name, dram_w in (("wk", wkT_d), ("wv", wvT_d), ("wq", wqT_d)):
            w = p_w.tile([P, DT, D], BF, tag="w", name=name)
            wr = dram_w.rearrange("(dt p) e -> dt p e", p=P)
            for dt in range(DT):
                nc.sync.dma_start(out=w[:, dt, :], in_=wr[dt])
            ws[name] = w

        # ---- K^T own-half projection: kt_tmp[e, k] = Wk @ t^T + bk ----
        kt_tmp = p_tmp.tile([P, DT, NOWN], BF, tag="tmp", name="kt_tmp")
        for et in range(DT):
            for nch in range(NOWN // CH):
                ps = p_ps.tile([P, CH], F32, tag="acc", name="ps_k")
                for dt in range(DT):
                    nc.tensor.matmul(
                        ps,
                        lhsT=ws["wk"][:, dt, et * P:(et + 1) * P],
                        rhs=tq[:, dt, nch * CH:(nch + 1) * CH],
                        start=(dt == 0), stop=(dt == DT - 1),
                    )
                nc.scalar.activation(
                    out=kt_tmp[:, et, nch * CH:(nch + 1) * CH], in_=ps,
                    func=AF.Identity, bias=bk_sb[:, et:et + 1], scale=1.0,
                )
        for et in range(DT):
            nc.sync.dma_start(out=cc_in_k[et * P:(et + 1) * P, :],
                              in_=kt_tmp[:, et, :])
        if no_cc:
            for r in range(2):
                nc.sync.dma_start(out=cc_out_k[r], in_=cc_in_k[:])
        else:
            nc.gpsimd.collective_compute(
                "AllGather", mybir.AluOpType.bypass, replica_groups=GROUPS,
                ins=[cc_in_k.opt()], outs=[cc_out_k.opt()],
            )

        # ---- V own-half projection (keys on partitions): v = t @ Wv^T ----
        v_tmp = p_tmp.tile([P, DT, D], BF, tag="tmp", name="v_tmp")
        for nt in range(DT):
            for ech in range(D // CH):
                ps = p_ps.tile([P, CH], F32, tag="acc", name="ps_v")
                for dt in range(DT):
                    nc.tensor.matmul(
                        ps,
                        lhsT=tq[:, dt, nt * P:(nt + 1) * P],
                        rhs=ws["wv"][:, dt, ech * CH:(ech + 1) * CH],
                        start=(dt == 0), stop=(dt == DT - 1),
                    )
                nc.vector.tensor_copy(out=v_tmp[:, nt, ech * CH:(ech + 1) * CH],
                                      in_=ps)
        for nt in range(DT):
            nc.sync.dma_start(out=cc_in_v[nt * P:(nt + 1) * P, :],
                              in_=v_tmp[:, nt, :])
        if no_cc:
            for r in range(2):
                nc.sync.dma_start(out=cc_out_v[r], in_=cc_in_v[:])
        else:
            nc.gpsimd.collective_compute(
                "AllGather", mybir.AluOpType.bypass, replica_groups=GROUPS,
                ins=[cc_in_v.opt()], outs=[cc_out_v.opt()],
            )

        # ---- Q^T projection ----
        qt = p_qt.tile([P, DT, NQ], BF, tag="qt")
        for et in range(DT):
            for nch in range(NQ // CH):
                ps = p_ps.tile([P, CH], F32, tag="acc", name="ps_q")
                for dt in range(DT):
                    nc.tensor.matmul(
                        ps,
                        lhsT=ws["wq"][:, dt, et * P:(et + 1) * P],
                        rhs=tq[:, dt, nch * CH:(nch + 1) * CH],
                        start=(dt == 0), stop=(dt == DT - 1),
                    )
                nc.scalar.activation(
                    out=qt[:, et, nch * CH:(nch + 1) * CH], in_=ps,
                    func=AF.Identity, bias=bq_sb[:, et:et + 1], scale=1.0,
                )

        # ---- gather results back to SBUF ----
        kt = p_kt.tile([P, DT, NKEYS], BF, tag="kt")
        for r in range(2):
            for et in range(DT):
                nc.sync.dma_start(
                    out=kt[:, et, r * NOWN:(r + 1) * NOWN],
                    in_=cc_out_k[r, et * P:(et + 1) * P, :],
                )
        v_sb = p_v.tile([P, NT, D], BF, tag="v")
        for r in range(2):
            for ntl in range(DT):
                nc.sync.dma_start(
                    out=v_sb[:, r * DT + ntl, :],
                    in_=cc_out_v[r, ntl * P:(ntl + 1) * P, :],
                )

        # ---- scores + exp: E^T[k, q] = exp(K @ Q^T) ----
        e_tiles = [p_w.tile([P, DT, NQ], BF, tag="w", name=f"e{i}")
                   for i in range(NT // DT)]

        def e_slice(kt_i, sl):
            return e_tiles[kt_i // DT][:, kt_i % DT, sl]

        for kt_i in range(NT):
            for qch in range(NQ // CH):
                ps = p_ps.tile([P, CH], F32, tag="acc", name="ps_s")
                for et in range(DT):
                    nc.tensor.matmul(
                        ps,
                        lhsT=kt[:, et, kt_i * P:(kt_i + 1) * P],
                        rhs=qt[:, et, qch * CH:(qch + 1) * CH],
                        start=(et == 0), stop=(et == DT - 1),
                    )
                nc.scalar.activation(
                    out=e_slice(kt_i, slice(qch * CH, (qch + 1) * CH)), in_=ps,
                    func=AF.Exp,
                )

        # ---- attention output: out[q, :] = (E @ [V | 1]), normalize, +bv ----
        for qt_i in range(QT):
            pss = [p_ps.tile([P, CH], F32, tag="acc", name=f"av{qt_i}_{i}")
                   for i in range(D // CH)]
            psd = p_psd.tile([P, 1], F32, tag="dsum", name="psd")
            for kt_i in range(NT):
                lhsT = e_slice(kt_i, slice(qt_i * P, (qt_i + 1) * P))
                for ech in range(D // CH):
                    nc.tensor.matmul(
                        pss[ech], lhsT=lhsT,
                        rhs=v_sb[:, kt_i, ech * CH:(ech + 1) * CH],
                        start=(kt_i == 0), stop=(kt_i == NT - 1),
                    )
                nc.tensor.matmul(
                    psd, lhsT=lhsT, rhs=ones_sb,
                    start=(kt_i == 0), stop=(kt_i == NT - 1),
                )
            rd = p_rd.tile([P, 1], F32, tag="rd", name="rd")
            nc.vector.reciprocal(out=rd, in_=psd)
            o_t = p_out.tile([P, D], F32, tag="out", name="o_t")
            for ech in range(D // CH):
                nc.scalar.activation(
                    out=o_t[:, ech * CH:(ech + 1) * CH], in_=pss[ech],
                    func=AF.Copy, scale=rd,
                )
            nc.vector.tensor_add(o_t, o_t, bv_sb)
            nc.sync.dma_start(out=out_d[qt_i * P:(qt_i + 1) * P, :], in_=o_t)


def prepare_in_maps(t, W_qkv, b_qkv):
    t = np.asarray(t, dtype=np.float32)
    W = np.asarray(W_qkv, dtype=np.float32)
    b = np.asarray(b_qkv, dtype=np.float32)
    B, N, _ = t.shape
    assert (B, N) == (4, 2048)

    bf16 = ml_dtypes.bfloat16
    scale = 1.0 / math.sqrt(D)
    wqT = np.ascontiguousarray((W[:D].T * scale).astype(bf16))       # [d, e]
    wkT = np.ascontiguousarray(W[D:2 * D].T.astype(bf16))
    wvT = np.ascontiguousarray(W[2 * D:].T.astype(bf16))
    bq = np.ascontiguousarray((b[:D] * scale).astype(np.float32).reshape(DT, P))
    bk = np.ascontiguousarray(b[D:2 * D].reshape(DT, P))
    bv = np.ascontiguousarray(b[2 * D:])

    t_bf = t.astype(bf16)
    in_maps = []
    for core in range(8):
        bi, h = core // 2, core % 2
        tq = np.ascontiguousarray(t_bf[bi].T[:, h * NQ:(h + 1) * NQ])
        in_maps.append({
            "tq": tq, "wqT": wqT, "wkT": wkT, "wvT": wvT,
            "bq": bq, "bk": bk, "bv": bv,
        })
    return in_maps


def get_nc(n_reps=1, no_cc=False):
    key = ("nc", n_reps, no_cc)
    if key not in _CACHE:
        _CACHE[key] = _build_nc(n_reps, no_cc=no_cc)
    return _CACHE[key]


def kernel(t, W_qkv, b_qkv):
    global LAST_RESULTS
    in_maps = prepare_in_maps(t, W_qkv, b_qkv)
    nc = get_nc()

    res = run_bass_kernel_spmd(
        nc, in_maps, core_ids=list(range(8)),
        trace=bool(int(os.environ.get("ATT_TRACE", "0") or "0")),
    )
    LAST_RESULTS = res

    out = np.empty((4, 2048, D), dtype=np.float32)
    for core in range(8):
        bi, h = core // 2, core % 2
        out[bi, h * NQ:(h + 1) * NQ, :] = res.results[core]["out"]
    return out


# revision 28
# speedup vs baseline: 1.0105x; 1.0105x over previous
"""Fused self-attention (QKV projection + softmax attention) on 8 trn2 cores.

Problem shapes: t [4, 2048, 1024] f32, W_qkv [3072, 1024], b_qkv [3072].
out = softmax((t@Wq.T+bq) @ (t@Wk.T+bk).T / sqrt(1024)) @ (t@Wv.T+bv),
per batch; output [4, 2048, 1024] f32.

Sharding: 8 cores = (batch b in 0..3) x (query-half h in 0..1). Each core:
  - receives t[b].T restricted to its own query-half columns,
  - projects Q^T, and K^T / V for its own 1024 keys only,
  - pairwise-AllGathers K^T then V with its batch partner (keys end up in
    natural order on both cores; the program stays rank-agnostic),
  - S^T = K @ Q^T over all 2048 keys, E^T = exp(S^T) (no max subtraction:
    |logits| < ~6 for this input distribution),
  - out = (E^T).T @ [V | ones] -> unnormalized out + row-sum D,
  - out = out * (1/D) + bv.
All matmuls bf16 with fp32 PSUM accumulation; the 1/sqrt(d_k) scale is folded
into Wq/bq on the host. Per-core matmul work is 15.05 GFLOP = total/8, the
parallel minimum; the two collectives overlap with Q projection and scores.

Measured (8x trn2, axon): ~250 us per execution (12-rep vs 4-rep NEFF
marginal, dispatch-cancelled), rel err vs fp32 reference 0.46%. The 896
N=512 matmuls at the hardware's measured 254.5 ns/MM envelope give a
~228 us floor for this decomposition; collectives and all DVE/ACT/DMA
work are hidden under the PE stream.
"""

import math
import os
from contextlib import ExitStack

import numpy as np
import ml_dtypes

import concourse.bass as bass
import concourse.tile as tile
from concourse import bacc, mybir
from concourse.bass_utils import run_bass_kernel_spmd

P = 128
D = 1024          # d_model = d_k = d_v
NKEYS = 2048      # keys per batch (after gather)
NOWN = 1024       # keys projected per core
NQ = 1024         # queries per core
DT = D // P       # 8 contraction tiles
NT = NKEYS // P   # 16 key tiles
QT = NQ // P      # 8 query tiles
CH = 512          # moving-operand chunk (one PSUM bank of fp32)
BF = mybir.dt.bfloat16
F32 = mybir.dt.float32
AF = mybir.ActivationFunctionType
GROUPS = [[0, 1], [2, 3], [4, 5], [6, 7]]

_CACHE = {}
LAST_RESULTS = None


def _build_nc(n_reps=1, no_cc=False):
    nc = bacc.Bacc("TRN2", target_bir_lowering=False, debug=False, num_devices=8)

    tq_d = nc.dram_tensor("tq", [D, NOWN], BF, kind="ExternalInput").ap()
    wqT_d = nc.dram_tensor("wqT", [D, D], BF, kind="ExternalInput").ap()
    wkT_d = nc.dram_tensor("wkT", [D, D], BF, kind="ExternalInput").ap()
    wvT_d = nc.dram_tensor("wvT", [D, D], BF, kind="ExternalInput").ap()
    bq_d = nc.dram_tensor("bq", [DT, P], F32, kind="ExternalInput").ap()
    bk_d = nc.dram_tensor("bk", [DT, P], F32, kind="ExternalInput").ap()
    bv_d = nc.dram_tensor("bv", [D], F32, kind="ExternalInput").ap()
    out_d = nc.dram_tensor("out", [NQ, D], F32, kind="ExternalOutput").ap()

    with tile.TileContext(nc) as tc, ExitStack() as ctx:
        consts = ctx.enter_context(tc.tile_pool(name="consts", bufs=1))
        p_rd = ctx.enter_context(tc.tile_pool(name="p_rd", bufs=2))
        p_t = ctx.enter_context(tc.tile_pool(name="p_t", bufs=1))
        p_w = ctx.enter_context(tc.tile_pool(name="p_w", bufs=3))
        p_tmp = ctx.enter_context(tc.tile_pool(name="p_tmp", bufs=2))
        p_kt = ctx.enter_context(tc.tile_pool(name="p_kt", bufs=1))
        p_qt = ctx.enter_context(tc.tile_pool(name="p_qt", bufs=1))
        p_v = ctx.enter_context(tc.tile_pool(name="p_v", bufs=1))
        p_out = ctx.enter_context(tc.tile_pool(name="p_out", bufs=3))
        p_ps = ctx.enter_context(tc.tile_pool(name="p_ps", bufs=6, space="PSUM"))
        p_psd = ctx.enter_context(tc.tile_pool(name="p_psd", bufs=2, space="PSUM"))
        dram = ctx.enter_context(tc.tile_pool(name="dram", bufs=1, space="DRAM"))

        # ---- constants (loaded once) ----
        bq_sb = consts.tile([P, DT], F32, tag="bq")
        nc.sync.dma_start(out=bq_sb, in_=bq_d.rearrange("a p -> p a"))
        bk_sb = consts.tile([P, DT], F32, tag="bk")
        nc.sync.dma_start(out=bk_sb, in_=bk_d.rearrange("a p -> p a"))
        bv_sb = consts.tile([P, D], F32, tag="bv")
        nc.sync.dma_start(
            out=bv_sb,
            in_=bass.AP(tensor=bv_d.tensor, offset=bv_d.offset,
                        ap=[[0, P]] + list(bv_d.ap)),
        )
        ones_sb = consts.tile([P, 1], BF, tag="ones")
        nc.vector.memset(ones_sb, 1.0)

        for _rep in range(n_reps):
            _emit_body(nc, tc, locals(), no_cc=no_cc)

    nc.compile()
    return nc


def _emit_body(nc, tc, env, no_cc=False):
    consts = env["consts"]; p_rd = env["p_rd"]; p_t = env["p_t"]
    p_w = env["p_w"]; p_tmp = env["p_tmp"]; p_kt = env["p_kt"]
    p_qt = env["p_qt"]; p_v = env["p_v"]; p_out = env["p_out"]
    p_ps = env["p_ps"]; p_psd = env["p_psd"]; dram = env["dram"]
    bq_sb = env["bq_sb"]; bk_sb = env["bk_sb"]; bv_sb = env["bv_sb"]
    ones_sb = env["ones_sb"]
    tq_d = env["tq_d"]; wqT_d = env["wqT_d"]; wkT_d = env["wkT_d"]
    wvT_d = env["wvT_d"]; out_d = env["out_d"]

    if True:
        cc_in_k = dram.tile([D, NOWN], BF, tag="cik", name="cc_in_k")
        cc_out_k = dram.tile([2, D, NOWN], BF, tag="cok", name="cc_out_k")
        cc_in_v = dram.tile([NOWN, D], BF, tag="civ", name="cc_in_v")
        cc_out_v = dram.tile([2, NOWN, D], BF, tag="cov", name="cc_out_v")

        # ---- input loads ----
        tq = p_t.tile([P, DT, NOWN], BF, tag="tq", name="tq")
        tq_r = tq_d.rearrange("(dt p) n -> dt p n", p=P)
        for dt in range(DT):
            nc.sync.dma_start(out=tq[:, dt, :], in_=tq_r[dt])

        ws = {}
        for name, dram_w in (("wk", wkT_d), ("wv", wvT_d), ("wq", wqT_d)):
            w = p_w.tile([P, DT, D], BF, tag="w", name=name)
            wr = dram_w.rearrange("(dt p) e -> dt p e", p=P)
            for dt in range(DT):
                nc.sync.dma_start(out=w[:, dt, :], in_=wr[dt])
            ws[name] = w

        # ---- K^T own-half projection: kt_tmp[e, k] = Wk @ t^T + bk ----
        kt_tmp = p_tmp.tile([P, DT, NOWN], BF, tag="tmp", name="kt_tmp")
        for et in range(DT):
            for nch in range(NOWN // CH):
                ps = p_ps.tile([P, CH], F32, tag="acc", name="ps_k")
                for dt in range(DT):
                    nc.tensor.matmul(
                        ps,
                        lhsT=ws["wk"][:, dt, et * P:(et + 1) * P],
                        rhs=tq[:, dt, nch * CH:(nch + 1) * CH],
                        start=(dt == 0), stop=(dt == DT - 1),
                    )
                nc.scalar.activation(
                    out=kt_tmp[:, et, nch * CH:(nch + 1) * CH], in_=ps,
                    func=AF.Identity, bias=bk_sb[:, et:et + 1], scale=1.0,
                )
        for et in range(DT):
            nc.sync.dma_start(out=cc_in_k[et * P:(et + 1) * P, :],
                              in_=kt_tmp[:, et, :])
        if no_cc:
            for r in range(2):
                nc.sync.dma_start(out=cc_out_k[r], in_=cc_in_k[:])
        else:
            nc.gpsimd.collective_compute(
                "AllGather", mybir.AluOpType.bypass, replica_groups=GROUPS,
                ins=[cc_in_k.opt()], outs=[cc_out_k.opt()],
            )

        # ---- V own-half projection (keys on partitions): v = t @ Wv^T ----
        v_tmp = p_tmp.tile([P, DT, D], BF, tag="tmp", name="v_tmp")
        for nt in range(DT):
            for ech in range(D // CH):
                ps = p_ps.tile([P, CH], F32, tag="acc", name="ps_v")
                for dt in range(DT):
                    nc.tensor.matmul(
                        ps,
                        lhsT=tq[:, dt, nt * P:(nt + 1) * P],
                        rhs=ws["wv"][:, dt, ech * CH:(ech + 1) * CH],
                        start=(dt == 0), stop=(dt == DT - 1),
                    )
                nc.vector.tensor_copy(out=v_tmp[:, nt, ech * CH:(ech + 1) * CH],
                                      in_=ps)
        for nt in range(DT):
            nc.sync.dma_start(out=cc_in_v[nt * P:(nt + 1) * P, :],
                              in_=v_tmp[:, nt, :])
        if no_cc:
            for r in range(2):
                nc.sync.dma_start(out=cc_out_v[r], in_=cc_in_v[:])
        else:
            nc.gpsimd.collective_compute(
                "AllGather", mybir.AluOpType.bypass, replica_groups=GROUPS,
                ins=[cc_in_v.opt()], outs=[cc_out_v.opt()],
            )

        # ---- gather results back to SBUF (queued right after the CCs) ----
        kt = p_kt.tile([P, DT, NKEYS], BF, tag="kt")
        for r in range(2):
            for et in range(DT):
                nc.sync.dma_start(
                    out=kt[:, et, r * NOWN:(r + 1) * NOWN],
                    in_=cc_out_k[r, et * P:(et + 1) * P, :],
                )
        v_sb = p_v.tile([P, NT, D], BF, tag="v")
        for r in range(2):
            for ntl in range(DT):
                nc.sync.dma_start(
                    out=v_sb[:, r * DT + ntl, :],
                    in_=cc_out_v[r, ntl * P:(ntl + 1) * P, :],
                )

        # ---- Q^T projection ----
        qt = p_qt.tile([P, DT, NQ], BF, tag="qt")
        for et in range(DT):
            for nch in range(NQ // CH):
                ps = p_ps.tile([P, CH], F32, tag="acc", name="ps_q")
                for dt in range(DT):
                    nc.tensor.matmul(
                        ps,
                        lhsT=ws["wq"][:, dt, et * P:(et + 1) * P],
                        rhs=tq[:, dt, nch * CH:(nch + 1) * CH],
                        start=(dt == 0), stop=(dt == DT - 1),
                    )
                nc.scalar.activation(
                    out=qt[:, et, nch * CH:(nch + 1) * CH], in_=ps,
                    func=AF.Identity, bias=bq_sb[:, et:et + 1], scale=1.0,
                )

        # ---- scores + exp: E^T[k, q] = exp(K @ Q^T) ----
        e_tiles = [p_w.tile([P, DT, NQ], BF, tag="w", name=f"e{i}")
                   for i in range(NT // DT)]

        def e_slice(kt_i, sl):
            return e_tiles[kt_i // DT][:, kt_i % DT, sl]

        for kt_i in range(NT):
            for qch in range(NQ // CH):
                ps = p_ps.tile([P, CH], F32, tag="acc", name="ps_s")
                for et in range(DT):
                    nc.tensor.matmul(
                        ps,
                        lhsT=kt[:, et, kt_i * P:(kt_i + 1) * P],
                        rhs=qt[:, et, qch * CH:(qch + 1) * CH],
                        start=(et == 0), stop=(et == DT - 1),
                    )
                nc.scalar.activation(
                    out=e_slice(kt_i, slice(qch * CH, (qch + 1) * CH)), in_=ps,
                    func=AF.Exp,
                )

        # ---- attention output: out[q, :] = (E @ [V | 1]), normalize, +bv ----
        for qt_i in range(QT):
            pss = [p_ps.tile([P, CH], F32, tag="acc", name=f"av{qt_i}_{i}")
                   for i in range(D // CH)]
            psd = p_psd.tile([P, 1], F32, tag="dsum", name="psd")
            for kt_i in range(NT):
                lhsT = e_slice(kt_i, slice(qt_i * P, (qt_i + 1) * P))
                for ech in range(D // CH):
                    nc.tensor.matmul(
                        pss[ech], lhsT=lhsT,
                        rhs=v_sb[:, kt_i, ech * CH:(ech + 1) * CH],
                        start=(kt_i == 0), stop=(kt_i == NT - 1),
                    )
                nc.tensor.matmul(
                    psd, lhsT=lhsT, rhs=ones_sb,
                    start=(kt_i == 0), stop=(kt_i == NT - 1),
                )
            rd = p_rd.tile([P, 1], F32, tag="rd", name="rd")
            nc.vector.reciprocal(out=rd, in_=psd)
            o_t = p_out.tile([P, D], F32, tag="out", name="o_t")
            for ech in range(D // CH):
                nc.scalar.activation(
                    out=o_t[:, ech * CH:(ech + 1) * CH], in_=pss[ech],
                    func=AF.Copy, scale=rd,
                )
            nc.vector.tensor_add(o_t, o_t, bv_sb)
            nc.sync.dma_start(out=out_d[qt_i * P:(qt_i + 1) * P, :], in_=o_t)


def prepare_in_maps(t, W_qkv, b_qkv):
    t = np.asarray(t, dtype=np.float32)
    W = np.asarray(W_qkv, dtype=np.float32)
    b = np.asarray(b_qkv, dtype=np.float32)
    B, N, _ = t.shape
    assert (B, N) == (4, 2048)

    bf16 = ml_dtypes.bfloat16
    scale = 1.0 / math.sqrt(D)
    wqT = np.ascontiguousarray((W[:D].T * scale).astype(bf16))       # [d, e]
    wkT = np.ascontiguousarray(W[D:2 * D].T.astype(bf16))
    wvT = np.ascontiguousarray(W[2 * D:].T.astype(bf16))
    bq = np.ascontiguousarray((b[:D] * scale).astype(np.float32).reshape(DT, P))
    bk = np.ascontiguousarray(b[D:2 * D].reshape(DT, P))
    bv = np.ascontiguousarray(b[2 * D:])

    t_bf = t.astype(bf16)
    in_maps = []
    for core in range(8):
        bi, h = core // 2, core % 2
        tq = np.ascontiguousarray(t_bf[bi].T[:, h * NQ:(h + 1) * NQ])
        in_maps.append({
            "tq": tq, "wqT": wqT, "wkT": wkT, "wvT": wvT,
            "bq": bq, "bk": bk, "bv": bv,
        })
    return in_maps


def get_nc(n_reps=1, no_cc=False):
    key = ("nc", n_reps, no_cc)
    if key not in _CACHE:
        _CACHE[key] = _build_nc(n_reps, no_cc=no_cc)
    return _CACHE[key]


def kernel(t, W_qkv, b_qkv):
    global LAST_RESULTS
    in_maps = prepare_in_maps(t, W_qkv, b_qkv)
    nc = get_nc()

    res = run_bass_kernel_spmd(
        nc, in_maps, core_ids=list(range(8)),
        trace=bool(int(os.environ.get("ATT_TRACE", "0") or "0")),
    )
    LAST_RESULTS = res

    out = np.empty((4, 2048, D), dtype=np.float32)
    for core in range(8):
        bi, h = core // 2, core % 2
        out[bi, h * NQ:(h + 1) * NQ, :] = res.results[core]["out"]
    return out
